# revision 18
# baseline (speedup 1.0000x reference)
"""Multi-head causal attention with RoPE on 8 Trainium2 NeuronCores.

Sharding: core c = 2*b + g handles batch b (of 4) and head-group g (of 2,
8 heads each).  Each core computes its 8 heads' attention and a partial
output projection (against its column-slice of wo); the host sums the two
partials per batch.

v2 layout notes (on top of the v1 fp16/triangle-optimal design):
 - Phase A interleaves q/k/v chains per head-pair (q0,k0,v0,q1,...) so the
   PE stream never waits on the DVE RoPE chains; QKV PSUM is evicted to
   fp16 SBUF by ACT and the whole RoPE chain (shuffle/mul/mul/add) runs in
   fp16, where the DVE tensor ops hit the 2x packed mode.
 - Weight DMAs (wq,wk,wv) all ride the sync queue in k-chunks; x(sb0)
   rides scalar+gpsimd split in k-chunks so the first matmul starts ~3us
   in; wo prefetches on gpsimd during phase A.
 - Phase B is ONE continuous software pipeline across all (J, hp) groups:
   scores(slot)+exp stream in global slot order, attnV trails DEPTH slots
   behind, normalize/outproj are spliced in as their deps complete.  The
   PE queue never drains at group boundaries (the v1 per-group drain cost
   ~1.3us + a DVFS ramp reset each).
 - attnV for the odd head of each pair writes PSUM partitions 63:128
   ([1|V] stationary layout: denominator row 63, dims 64:127), so the
   normalized output lands directly in oT's upper half -- the v1
   DRAM-shift DMAs (and their gpsimd queue congestion) are gone.
 - 1/den: per-h2 32-block stream-transpose + strided reciprocal +
   transpose back (~1.2us DVE), then a DRAM-bounce broadcast on the sync
   queue whose latency hides under the one-group outproj deferral.  The
   drain-path (last group) broadcasts via a PE outer product instead.
"""

import sys

sys.path.insert(0, "/opt/trn_rl_repo")

import numpy as np

D_MODEL = 1024
NUM_HEADS = 16
D_K = 64
B_FULL, S = 4, 2048
THETA = 10000.0
N_CORES = 8
H_CORE = 8  # heads per core
HP = 4      # head pairs per core
SB = 4      # 512-wide s-blocks
ST = 16     # 128-wide s-tiles
KT = 8      # 128-deep k-tiles over D_MODEL
MASK_NEG = -30000.0  # causal mask additive constant (fp16-representable)
DEPTH = 7   # attnV lag (slots) behind scores in the phase-B pipeline

# stream_shuffle applies its 32-entry mask within each 32-partition block:
# this swaps the two 16-row halves of every block.
SHUF16 = list(range(16, 32)) + list(range(0, 16))

_CACHE = {}


def _build_module(mm="float16"):
    import concourse.bacc as bacc
    import concourse.tile as tile
    from concourse import mybir
    from contextlib import ExitStack

    P = 128
    FP32 = mybir.dt.float32
    M16 = getattr(mybir.dt, mm)      # qk-side matmul dtype
    MR = mybir.dt.float32r          # outproj matmul dtype
    EXP = mybir.ActivationFunctionType.Exp

    nc = bacc.Bacc("TRN2", target_bir_lowering=False, debug=False,
                   num_devices=N_CORES)

    xT = nc.dram_tensor("xT", [D_MODEL, S], M16, kind="ExternalInput")
    wqT = nc.dram_tensor("wqT", [D_MODEL, 512], M16, kind="ExternalInput")
    wkT = nc.dram_tensor("wkT", [D_MODEL, 512], M16, kind="ExternalInput")
    wvT = nc.dram_tensor("wvT", [D_MODEL, 512], M16, kind="ExternalInput")
    woT = nc.dram_tensor("woT", [512, D_MODEL], MR, kind="ExternalInput")
    cosT = nc.dram_tensor("cosT", [P, S], M16, kind="ExternalInput")
    sinT = nc.dram_tensor("sinT", [P, S], M16, kind="ExternalInput")
    maskA = nc.dram_tensor("maskA", [P, P], M16, kind="ExternalInput")
    outD = nc.dram_tensor("out", [S, D_MODEL], FP32, kind="ExternalOutput")
    # denominator-reciprocal bounce buffer: one row per (J, hp, head)
    scr = nc.dram_tensor("scr", [SB, HP, 2, 512], FP32)

    xT3 = xT.rearrange("(ko p) s -> p ko s", p=P)
    wqT3 = wqT.rearrange("(ko p) m -> p ko m", p=P)
    wkT3 = wkT.rearrange("(ko p) m -> p ko m", p=P)
    wvT3 = wvT.rearrange("(ko p) m -> p ko m", p=P)
    woT3 = woT.rearrange("(t p) n -> p t n", p=P)

    with tile.TileContext(nc) as tc:
        with ExitStack() as ctx:
            const_pool = ctx.enter_context(tc.tile_pool(name="const", bufs=1))
            qk_pool = ctx.enter_context(tc.tile_pool(name="qk", bufs=1))
            v_pool = ctx.enter_context(tc.tile_pool(name="vp", bufs=1))
            wopool = ctx.enter_context(tc.tile_pool(name="wo", bufs=1))
            # ex tiles and score-PSUM live across both phases: J0 score
            # slots are hoisted into phase A (sb1) to pre-seed the phase-B
            # pipeline, so ACT has a head start on the exp stream
            epool = ctx.enter_context(tc.tile_pool(name="expp", bufs=26))
            psS = ctx.enter_context(
                tc.tile_pool(name="psS", bufs=2, space="PSUM"))

            # 0/1 causal mask (1 where query >= key), multiplied into the
            # fp16 ex tile on DVE
            mask01_sb = const_pool.tile([P, P], M16, name="mask01_sb")
            # ones rows for the drain-path PE broadcast of 1/den
            ones64 = const_pool.tile([P, 64], FP32, name="ones64")
            nc.vector.memset(ones64[:, :], 1.0)

            qt = [qk_pool.tile([P, S], M16, tag=f"qt{i}", name=f"qt{i}")
                  for i in range(HP)]
            kt = [qk_pool.tile([P, S], M16, tag=f"kt{i}", name=f"kt{i}")
                  for i in range(HP)]
            # per head: [V(64) | 1] -- the ones column (64) makes the
            # attnV matmul emit the softmax denominator as PSUM row 64.
            v_sb = v_pool.tile([P, ST, H_CORE, 65], M16)
            # only the ones-columns need initializing (the dim columns are
            # overwritten by the v evictions); a full-tile memset costs 7us
            # of DVE right when the first RoPE chains need the engine
            nc.vector.memset(v_sb[:, :, :, 64:65], 1.0)

            wo_sb = wopool.tile([P, 4, D_MODEL], MR)

            def nlo_of(I, J):
                r = I - 4 * J
                return 128 * r if r >= 0 else 0

            def scores(J, hp, I):
                nlo = nlo_of(I, J)
                ks = slice(I * 128, (I + 1) * 128)
                qs = slice(J * 512 + nlo, (J + 1) * 512)
                diag = I - 4 * J >= 0
                ps = psS.tile([P, 2, 512], FP32, tag="psS", name="psS")
                for h2 in range(2):
                    hrows = slice(h2 * 64, (h2 + 1) * 64)
                    nc.tensor.matmul(
                        ps[:, h2, nlo:],
                        kt[hp][hrows, ks],
                        qt[hp][hrows, qs],
                        start=True, stop=True,
                    )
                ex = epool.tile([P, 2, 512], M16, tag="ex", name="ex")
                nc.scalar.activation(ex[:, :, nlo:], ps[:, :, nlo:],
                                     EXP, scale=0.125)
                if diag:
                    # causal corner: zero ex where query < key (fp16 SBUF
                    # elementwise hits the DVE 2x mode)
                    for h2 in range(2):
                        nc.vector.tensor_mul(
                            ex[:, h2, nlo:nlo + 128],
                            ex[:, h2, nlo:nlo + 128],
                            mask01_sb[:, :])
                return ex, nlo

            pend = []

            # ---------------- Phase A: QKV projections + RoPE -------------
            with ExitStack() as actx:
                wpool = actx.enter_context(tc.tile_pool(name="wts", bufs=1))
                xpool = actx.enter_context(tc.tile_pool(name="xs", bufs=2))
                cspool = actx.enter_context(tc.tile_pool(name="cs", bufs=1))
                rpool = actx.enter_context(tc.tile_pool(name="rope", bufs=2))
                psA = actx.enter_context(
                    tc.tile_pool(name="psA", bufs=2, space="PSUM"))

                cos_sb = cspool.tile([P, S], M16, tag="cos", name="cos_sb")
                sin_sb = cspool.tile([P, S], M16, tag="sin", name="sin_sb")

                # weights on the sync queue, split per k-tile so the first
                # matmuls start as soon as chunk 0 lands
                w_sb = {}
                for mode in ("q", "k", "v"):
                    w_sb[mode] = wpool.tile([P, KT, 512], M16, tag=f"w{mode}",
                                            name=f"w_{mode}")
                for wdram, mode in ((wqT3, "q"), (wkT3, "k"), (wvT3, "v")):
                    for k in range(KT):
                        nc.sync.dma_start(out=w_sb[mode][:, k, :],
                                          in_=wdram[:, k, :])
                xs0 = xpool.tile([P, KT, 1024], M16, tag="xs", name="xs")
                # 512-wide half-chunks, half 0 first: the opening chain
                # consumes [k, 0:512] for its first half, so it starts as
                # soon as the first 1KB/partition chunk lands
                for h in range(2):
                    hs = slice(h * 512, (h + 1) * 512)
                    for k in range(4):
                        nc.scalar.dma_start(out=xs0[:, k, hs],
                                            in_=xT3[:, k, hs])
                    for k in range(4, 8):
                        nc.gpsimd.dma_start(out=xs0[:, k, hs],
                                            in_=xT3[:, k, hs])
                nc.gpsimd.dma_start(out=mask01_sb[:], in_=maskA[:, :])
                nc.gpsimd.dma_start(out=cos_sb[:], in_=cosT[:, :])
                nc.gpsimd.dma_start(out=sin_sb[:], in_=sinT[:, :])
                # wo prefetch for phase B
                nc.gpsimd.dma_start(out=wo_sb[:], in_=woT3[:, :, :])

                for sb in range(2):
                    sbs = slice(sb * 1024, (sb + 1) * 1024)
                    if sb == 0:
                        xs = xs0
                    else:
                        xs = xpool.tile([P, KT, 1024], M16, tag="xs",
                                        name="xs")
                        nc.sync.dma_start(out=xs[:, 0:4, :],
                                          in_=xT3[:, 0:4, sbs])
                        nc.sync.dma_start(out=xs[:, 4:8, :],
                                          in_=xT3[:, 4:8, sbs])
                    cosv = cos_sb[:, sbs].rearrange("p (a b) -> p a b", a=2)
                    sinv = sin_sb[:, sbs].rearrange("p (a b) -> p a b", a=2)

                    def qk_chain(mode, hp):
                        dst = qt if mode == "q" else kt
                        hps = slice(hp * 128, (hp + 1) * 128)
                        ps = psA.tile([P, 2, 512], FP32, tag="pa", name="pa")
                        for half in range(2):
                            hs = slice(half * 512, (half + 1) * 512)
                            for k in range(KT):
                                nc.tensor.matmul(
                                    ps[:, half, :],
                                    w_sb[mode][:, k, hps],
                                    xs[:, k, hs],
                                    start=(k == 0), stop=(k == KT - 1),
                                )
                        s16 = rpool.tile([P, 2, 512], M16, tag="s16",
                                         name="s16")
                        nc.scalar.copy(s16[:], ps[:, :, :])
                        rot = rpool.tile([P, 2, 512], M16, tag="rot",
                                         name="rot")
                        nc.vector.stream_shuffle(rot[:], s16[:], mask=SHUF16)
                        t1 = rpool.tile([P, 2, 512], M16, tag="t1",
                                        name="t1")
                        nc.vector.tensor_mul(t1[:], s16[:], cosv)
                        t2 = rpool.tile([P, 2, 512], M16, tag="t2",
                                        name="t2")
                        nc.vector.tensor_mul(t2[:], rot[:], sinv)
                        nc.vector.tensor_add(
                            dst[hp][:, sbs].rearrange("p (a b) -> p a b",
                                                      a=2),
                            t1[:], t2[:])

                    def v_chain(stp2):
                        psv = psA.tile([P, 2, 512], FP32, tag="pa",
                                       name="psv")
                        for half in range(2):
                            st_off = slice((stp2 * 2 + half) * 128,
                                           (stp2 * 2 + half + 1) * 128)
                            for k in range(KT):
                                nc.tensor.matmul(
                                    psv[:, half, :],
                                    xs[:, k, st_off],
                                    w_sb["v"][:, k, :],
                                    start=(k == 0), stop=(k == KT - 1),
                                )
                        st0 = sb * 8 + stp2 * 2
                        pv = psv[:, :, :].rearrange(
                            "p a (h d) -> p a h d", h=8)
                        # DVE evict keeps the ACT queue clear for the
                        # hoisted-J0 exps at the phase boundary
                        nc.vector.tensor_copy(v_sb[:, st0:st0 + 2, :, 0:64],
                                              pv[:, :, :, :])

                    if sb == 0:
                        # defer v0/v1 past two extra qk chains: wv rides at
                        # the tail of the serial sync weight queue (~15us)
                        for hp in range(2):
                            qk_chain("q", hp)
                            qk_chain("k", hp)
                        for hp in range(2, HP):
                            v_chain(hp - 2)
                            qk_chain("q", hp)
                            qk_chain("k", hp)
                        v_chain(2)
                        v_chain(3)
                    else:
                        # sb1: v chains first (drains the ACT queue backlog
                        # before the exp stream), J0 score slots spliced
                        # between chains to pre-seed the phase-B pipeline
                        for hp in range(HP):
                            v_chain(hp)
                            qk_chain("q", hp)
                            for I in (0, 1):
                                ex, nlo = scores(0, hp, I)
                                pend.append((0, hp, ex, nlo, I))
                            qk_chain("k", hp)
                            for I in (2, 3):
                                ex, nlo = scores(0, hp, I)
                                pend.append((0, hp, ex, nlo, I))

            # ---------------- Phase B: attention ----------------
            ot_pool = ctx.enter_context(tc.tile_pool(name="otp", bufs=1))
            oT = [ot_pool.tile([P, S], MR, tag=f"oT{i}", name=f"oT{i}")
                  for i in range(HP)]

            with ExitStack() as bctx:
                rdpool = bctx.enter_context(tc.tile_pool(name="rdp", bufs=3))
                bcpool = bctx.enter_context(tc.tile_pool(name="bcp", bufs=3))
                opool = bctx.enter_context(tc.tile_pool(name="ostage",
                                                        bufs=2))
                psO = bctx.enter_context(
                    tc.tile_pool(name="psO", bufs=2, space="PSUM"))

                def attnv(po, hp, n_i, ex, nlo, I):
                    nc.tensor.matmul(
                        po[0][0:65, nlo:],
                        v_sb[:, I, hp * 2, :],
                        ex[:, 0, nlo:],
                        start=(I == 0), stop=(I == n_i - 1),
                    )
                    nc.tensor.matmul(
                        po[1][0:65, nlo:],
                        v_sb[:, I, hp * 2 + 1, :],
                        ex[:, 1, nlo:],
                        start=(I == 0), stop=(I == n_i - 1),
                    )

                def normalize(J, hp, po, last=False):
                    Js = slice(J * 512, (J + 1) * 512)
                    if last:
                        bcp = psS.tile([P, 2, 512], FP32, tag="psS",
                                       name="bcp")
                    for h2 in range(2):
                        # den row 64.  32-block stream-transpose spreads it
                        # over 32 partitions, reciprocal hits the one
                        # strided column holding it, transpose back.
                        tb = rdpool.tile([P, 512], FP32, tag="tb",
                                         name="tb")
                        nc.vector.transpose(tb[64:96, :], po[h2][64:96, :])
                        tb2 = rdpool.tile([P, 512], FP32, tag="tb2",
                                          name="tb2")
                        tbv = tb[64:96, :].rearrange("p (b q) -> p b q",
                                                     q=32)
                        tb2v = tb2[64:96, :].rearrange("p (b q) -> p b q",
                                                       q=32)
                        nc.vector.reciprocal(tb2v[:, :, 0:1],
                                             tbv[:, :, 0:1])
                        rd = rdpool.tile([P, 512], FP32, tag="rd",
                                         name="rd")
                        nc.vector.transpose(rd[64:96, :], tb2[64:96, :])
                        if last:
                            nc.tensor.matmul(
                                bcp[0:64, h2, :],
                                ones64[64:65, :],
                                rd[64:65, :],
                                start=True, stop=True,
                            )
                            bcs = bcpool.tile([P, 512], FP32,
                                              tag=f"bc{h2}", name="bcs")
                            nc.scalar.copy(bcs[0:64, :], bcp[0:64, h2, :])
                            bca = bcs[0:64, :]
                        else:
                            nc.sync.dma_start(out=scr[J, hp, h2, :],
                                              in_=rd[64:65, :])
                            bc = bcpool.tile([P, 512], FP32, tag=f"bc{h2}",
                                             name="bc")
                            nc.sync.dma_start(
                                out=bc[0:64, :],
                                in_=scr[J, hp, h2, :].partition_broadcast(
                                    64))
                            bca = bc[0:64, :]
                        if h2 == 0:
                            nc.vector.tensor_mul(
                                oT[hp][0:64, Js],
                                po[0][0:64, :], bca)
                        else:
                            # normalized evict lands at partitions 0-63; a
                            # sync-queue DMA shifts it into oT's upper half
                            tmp = rdpool.tile([P, 512], MR, tag="tmpb",
                                              name="tmpb")
                            nc.vector.tensor_mul(
                                tmp[0:64, :], po[1][0:64, :], bca)
                            nc.sync.dma_start(out=oT[hp][64:128, Js],
                                              in_=tmp[0:64, :])

                def outproj_st(J, i, qsel):
                    st = 4 * J + i
                    stp = slice(st * 128, (st + 1) * 128)
                    pc = psS.tile([P, 2, 512], FP32, tag="psS", name="pc")
                    for nb in range(2):
                        nbs = slice(nb * 512, (nb + 1) * 512)
                        for t in range(4):
                            nc.tensor.matmul(
                                pc[:, nb, :],
                                oT[t][:, stp],
                                wo_sb[:, t, nbs],
                                start=(t == 0), stop=(t == 3),
                            )
                    ob = opool.tile([P, 2, 512], FP32, tag="ob", name="ob")
                    nc.vector.tensor_copy(ob[:], pc[:, :])
                    nc.gpsimd.dma_start(
                        out=outD[stp, :],
                        in_=ob[:, :, :].rearrange("p a b -> p (a b)"))

                # ---- continuous global pipeline over remaining slots ----
                slots = [(J, hp, I)
                         for J in range(1, SB)
                         for hp in range(HP)
                         for I in range(4 * J + 4)]
                po_map = {}
                opq = []
                oq_count = 0

                def pop_attnv():
                    J, hp, ex, nlo, I = pend.pop(0)
                    g = (J, hp)
                    if I == 0:
                        po_map[g] = [psO.tile([P, 512], FP32, tag=f"po{h2}",
                                              name=f"po{h2}")
                                     for h2 in range(2)]
                    attnv(po_map[g], hp, 4 * J + 4, ex, nlo, I)
                    if I == 4 * J + 3:
                        normalize(J, hp, po_map.pop(g),
                                  last=(J == SB - 1 and hp == HP - 1))
                        if hp == HP - 1:
                            opq.extend((J, i) for i in range(4))

                for idx, (J, hp, I) in enumerate(slots):
                    ex, nlo = scores(J, hp, I)
                    pend.append((J, hp, ex, nlo, I))
                    if len(pend) > DEPTH:
                        pop_attnv()
                    if idx >= len(slots) - 12 and len(pend) > DEPTH:
                        pop_attnv()
                    if opq and idx % 6 == 5:
                        outproj_st(*opq.pop(0), oq_count)
                        oq_count += 1
                while pend:
                    pop_attnv()
                while opq:
                    outproj_st(*opq.pop(0), oq_count)
                    oq_count += 1

    nc.compile()
    return nc


def get_module(mm="float16"):
    if mm not in _CACHE:
        _CACHE[mm] = _build_module(mm)
    return _CACHE[mm]


def _head_perm():
    """Within-head dim permutation: 16-pair blocks [x1 x2 x1 x2]."""
    p = []
    for blk in range(2):
        base = blk * 32
        p += [2 * (base // 2 + i) for i in range(16)]       # x1 of pairs
        p += [2 * (base // 2 + i) + 1 for i in range(16)]   # x2 of pairs
    return np.array(p)


def prep_core_inputs(inputs, mm="float16"):
    import ml_dtypes
    mdt = {"float16": np.float16, "bfloat16": ml_dtypes.bfloat16}.get(
        mm, np.float32)
    x = np.asarray(inputs["x"], dtype=np.float32)
    tp = np.asarray(inputs["token_positions"])
    wq = np.asarray(inputs["wq"], dtype=np.float32)
    wk = np.asarray(inputs["wk"], dtype=np.float32)
    wv = np.asarray(inputs["wv"], dtype=np.float32)
    wo = np.asarray(inputs["wo"], dtype=np.float32)

    perm = _head_perm()
    qi = np.arange(128)[None, :]
    ki = np.arange(128)[:, None]
    mask01 = np.where(qi < ki, np.float32(0.0),
                      np.float32(1.0)).astype(mdt)

    freqs = 1.0 / THETA ** (np.arange(0, D_K, 2, dtype=np.float32) / D_K)

    in_maps = []
    for c in range(N_CORES):
        b, g = divmod(c, 2)
        rows = slice(g * 512, (g + 1) * 512)
        wq_g = wq[rows].reshape(H_CORE, D_K, D_MODEL)[:, perm, :]
        wk_g = wk[rows].reshape(H_CORE, D_K, D_MODEL)[:, perm, :]

        pos = tp[b].astype(np.float32)
        ang = freqs[:, None] * pos[None, :]          # [32, S]
        cos32, sin32 = np.cos(ang), np.sin(ang)
        # permuted row l: l%32 < 16 -> x1 of pair (l%32 + 16*(l//32)),
        #                 else x2 of the same pair; x1 rows get -sin.
        cos64 = np.concatenate([cos32[0:16], cos32[0:16],
                                cos32[16:32], cos32[16:32]], axis=0)
        sin64 = np.concatenate([-sin32[0:16], sin32[0:16],
                                -sin32[16:32], sin32[16:32]], axis=0)
        cosT = np.tile(cos64, (2, 1))
        sinT = np.tile(sin64, (2, 1))

        in_maps.append({
            "xT": np.ascontiguousarray(x[b].T).astype(mdt),
            "wqT": np.ascontiguousarray(
                wq_g.reshape(512, D_MODEL).T).astype(mdt),
            "wkT": np.ascontiguousarray(
                wk_g.reshape(512, D_MODEL).T).astype(mdt),
            "wvT": np.ascontiguousarray(wv[rows].T).astype(mdt),
            "woT": np.ascontiguousarray(wo[:, rows].T).astype(np.float32),
            "cosT": np.ascontiguousarray(cosT).astype(mdt),
            "sinT": np.ascontiguousarray(sinT).astype(mdt),
            "maskA": mask01,
        })
    return in_maps


DEFAULT_MM = "float16"


def kernel(**inputs):
    from concourse.bass_utils import run_bass_kernel_spmd

    import os
    mm = os.environ.get("KMM", DEFAULT_MM)
    nc = get_module(mm)
    in_maps = prep_core_inputs(inputs, mm)
    res = run_bass_kernel_spmd(nc, in_maps, core_ids=list(range(N_CORES)))
    out = np.empty((B_FULL, S, D_MODEL), np.float32)
    for b in range(B_FULL):
        out[b] = res.results[2 * b]["out"] + res.results[2 * b + 1]["out"]
    return out


# revision 19
# speedup vs baseline: 1.0199x; 1.0199x over previous
"""Multi-head causal attention with RoPE on 8 Trainium2 NeuronCores.

Sharding: core c = 2*b + g handles batch b (of 4) and head-group g (of 2,
8 heads each).  Each core computes its 8 heads' attention and a partial
output projection (against its column-slice of wo); the host sums the two
partials per batch.

v2 layout notes (on top of the v1 fp16/triangle-optimal design):
 - Phase A interleaves q/k/v chains per head-pair (q0,k0,v0,q1,...) so the
   PE stream never waits on the DVE RoPE chains; QKV PSUM is evicted to
   fp16 SBUF by ACT and the whole RoPE chain (shuffle/mul/mul/add) runs in
   fp16, where the DVE tensor ops hit the 2x packed mode.
 - Weight DMAs (wq,wk,wv) all ride the sync queue in k-chunks; x(sb0)
   rides scalar+gpsimd split in k-chunks so the first matmul starts ~3us
   in; wo prefetches on gpsimd during phase A.
 - Phase B is ONE continuous software pipeline across all (J, hp) groups:
   scores(slot)+exp stream in global slot order, attnV trails DEPTH slots
   behind, normalize/outproj are spliced in as their deps complete.  The
   PE queue never drains at group boundaries (the v1 per-group drain cost
   ~1.3us + a DVFS ramp reset each).
 - attnV for the odd head of each pair writes PSUM partitions 63:128
   ([1|V] stationary layout: denominator row 63, dims 64:127), so the
   normalized output lands directly in oT's upper half -- the v1
   DRAM-shift DMAs (and their gpsimd queue congestion) are gone.
 - 1/den: per-h2 32-block stream-transpose + strided reciprocal +
   transpose back (~1.2us DVE), then a DRAM-bounce broadcast on the sync
   queue whose latency hides under the one-group outproj deferral.  The
   drain-path (last group) broadcasts via a PE outer product instead.
"""

import sys

sys.path.insert(0, "/opt/trn_rl_repo")

import numpy as np

D_MODEL = 1024
NUM_HEADS = 16
D_K = 64
B_FULL, S = 4, 2048
THETA = 10000.0
N_CORES = 8
H_CORE = 8  # heads per core
HP = 4      # head pairs per core
SB = 4      # 512-wide s-blocks
ST = 16     # 128-wide s-tiles
KT = 8      # 128-deep k-tiles over D_MODEL
MASK_NEG = -30000.0  # causal mask additive constant (fp16-representable)
DEPTH = 7   # attnV lag (slots) behind scores in the phase-B pipeline

# stream_shuffle applies its 32-entry mask within each 32-partition block:
# this swaps the two 16-row halves of every block.
SHUF16 = list(range(16, 32)) + list(range(0, 16))

_CACHE = {}


def _build_module(mm="float16"):
    import concourse.bacc as bacc
    import concourse.tile as tile
    from concourse import mybir
    from contextlib import ExitStack

    P = 128
    FP32 = mybir.dt.float32
    M16 = getattr(mybir.dt, mm)      # qk-side matmul dtype
    MR = mybir.dt.float32r          # outproj matmul dtype
    EXP = mybir.ActivationFunctionType.Exp

    nc = bacc.Bacc("TRN2", target_bir_lowering=False, debug=False,
                   num_devices=N_CORES)

    xT = nc.dram_tensor("xT", [D_MODEL, S], M16, kind="ExternalInput")
    wqT = nc.dram_tensor("wqT", [D_MODEL, 512], M16, kind="ExternalInput")
    wkT = nc.dram_tensor("wkT", [D_MODEL, 512], M16, kind="ExternalInput")
    wvT = nc.dram_tensor("wvT", [D_MODEL, 512], M16, kind="ExternalInput")
    woT = nc.dram_tensor("woT", [512, D_MODEL], MR, kind="ExternalInput")
    cosT = nc.dram_tensor("cosT", [P, S], M16, kind="ExternalInput")
    sinT = nc.dram_tensor("sinT", [P, S], M16, kind="ExternalInput")
    maskA = nc.dram_tensor("maskA", [P, P], M16, kind="ExternalInput")
    outD = nc.dram_tensor("out", [S, D_MODEL], FP32, kind="ExternalOutput")
    # denominator-reciprocal bounce buffer: one row per (J, hp, head)
    scr = nc.dram_tensor("scr", [SB, HP, 2, 512], FP32)

    xT3 = xT.rearrange("(ko p) s -> p ko s", p=P)
    wqT3 = wqT.rearrange("(ko p) m -> p ko m", p=P)
    wkT3 = wkT.rearrange("(ko p) m -> p ko m", p=P)
    wvT3 = wvT.rearrange("(ko p) m -> p ko m", p=P)
    woT3 = woT.rearrange("(t p) n -> p t n", p=P)

    with tile.TileContext(nc) as tc:
        with ExitStack() as ctx:
            const_pool = ctx.enter_context(tc.tile_pool(name="const", bufs=1))
            qk_pool = ctx.enter_context(tc.tile_pool(name="qk", bufs=1))
            v_pool = ctx.enter_context(tc.tile_pool(name="vp", bufs=1))
            wopool = ctx.enter_context(tc.tile_pool(name="wo", bufs=1))
            # ex tiles and score-PSUM live across both phases: J0 score
            # slots are hoisted into phase A (sb1) to pre-seed the phase-B
            # pipeline, so ACT has a head start on the exp stream
            epool = ctx.enter_context(tc.tile_pool(name="expp", bufs=26))
            psS = ctx.enter_context(
                tc.tile_pool(name="psS", bufs=2, space="PSUM"))

            # 0/1 causal mask (1 where query >= key), multiplied into the
            # fp16 ex tile on DVE
            mask01_sb = const_pool.tile([P, P], M16, name="mask01_sb")
            nc.gpsimd.dma_start(out=mask01_sb[:], in_=maskA[:, :])
            # ones rows for the drain-path PE broadcast of 1/den
            ones64 = const_pool.tile([P, 64], FP32, name="ones64")
            nc.vector.memset(ones64[:, :], 1.0)

            qt = [qk_pool.tile([P, S], M16, tag=f"qt{i}", name=f"qt{i}")
                  for i in range(HP)]
            kt = [qk_pool.tile([P, S], M16, tag=f"kt{i}", name=f"kt{i}")
                  for i in range(HP)]
            # per head: [V(64) | 1] -- the ones column (64) makes the
            # attnV matmul emit the softmax denominator as PSUM row 64.
            v_sb = v_pool.tile([P, ST, H_CORE, 65], M16)
            # only the ones-columns need initializing (the dim columns are
            # overwritten by the v evictions); a full-tile memset costs 7us
            # of DVE right when the first RoPE chains need the engine
            nc.vector.memset(v_sb[:, :, :, 64:65], 1.0)

            wo_sb = wopool.tile([P, 4, D_MODEL], MR)

            def nlo_of(I, J):
                r = I - 4 * J
                return 128 * r if r >= 0 else 0

            def scores(J, hp, I):
                nlo = nlo_of(I, J)
                ks = slice(I * 128, (I + 1) * 128)
                qs = slice(J * 512 + nlo, (J + 1) * 512)
                diag = I - 4 * J >= 0
                ps = psS.tile([P, 2, 512], FP32, tag="psS", name="psS")
                for h2 in range(2):
                    hrows = slice(h2 * 64, (h2 + 1) * 64)
                    nc.tensor.matmul(
                        ps[:, h2, nlo:],
                        kt[hp][hrows, ks],
                        qt[hp][hrows, qs],
                        start=True, stop=True,
                    )
                ex = epool.tile([P, 2, 512], M16, tag="ex", name="ex")
                nc.scalar.activation(ex[:, :, nlo:], ps[:, :, nlo:],
                                     EXP, scale=0.125)
                if diag:
                    # causal corner: zero ex where query < key (fp16 SBUF
                    # elementwise hits the DVE 2x mode)
                    for h2 in range(2):
                        nc.vector.tensor_mul(
                            ex[:, h2, nlo:nlo + 128],
                            ex[:, h2, nlo:nlo + 128],
                            mask01_sb[:, :])
                return ex, nlo

            pend = []

            # ---------------- Phase A: QKV projections + RoPE -------------
            with ExitStack() as actx:
                wpool = actx.enter_context(tc.tile_pool(name="wts", bufs=1))
                xpool = actx.enter_context(tc.tile_pool(name="xs", bufs=2))
                cspool = actx.enter_context(tc.tile_pool(name="cs", bufs=1))
                rpool = actx.enter_context(tc.tile_pool(name="rope", bufs=2))
                psA = actx.enter_context(
                    tc.tile_pool(name="psA", bufs=2, space="PSUM"))

                cos_sb = cspool.tile([P, S], M16, tag="cos", name="cos_sb")
                sin_sb = cspool.tile([P, S], M16, tag="sin", name="sin_sb")

                # weights on the sync queue, split per k-tile so the first
                # matmuls start as soon as chunk 0 lands
                w_sb = {}
                for mode in ("q", "k", "v"):
                    w_sb[mode] = wpool.tile([P, KT, 512], M16, tag=f"w{mode}",
                                            name=f"w_{mode}")
                for wdram, mode in ((wqT3, "q"), (wkT3, "k"), (wvT3, "v")):
                    for k in range(KT):
                        nc.sync.dma_start(out=w_sb[mode][:, k, :],
                                          in_=wdram[:, k, :])
                xs0 = xpool.tile([P, KT, 1024], M16, tag="xs", name="xs")
                # 512-wide half-chunks, half 0 first: the opening chain
                # consumes [k, 0:512] for its first half, so it starts as
                # soon as the first 1KB/partition chunk lands
                for h in range(2):
                    hs = slice(h * 512, (h + 1) * 512)
                    for k in range(4):
                        nc.scalar.dma_start(out=xs0[:, k, hs],
                                            in_=xT3[:, k, hs])
                    for k in range(4, 8):
                        nc.gpsimd.dma_start(out=xs0[:, k, hs],
                                            in_=xT3[:, k, hs])
                nc.gpsimd.dma_start(out=cos_sb[:], in_=cosT[:, :])
                nc.gpsimd.dma_start(out=sin_sb[:], in_=sinT[:, :])
                # wo prefetch for phase B
                nc.gpsimd.dma_start(out=wo_sb[:], in_=woT3[:, :, :])

                for sb in range(2):
                    sbs = slice(sb * 1024, (sb + 1) * 1024)
                    if sb == 0:
                        xs = xs0
                    else:
                        xs = xpool.tile([P, KT, 1024], M16, tag="xs",
                                        name="xs")
                        nc.sync.dma_start(out=xs[:, 0:4, :],
                                          in_=xT3[:, 0:4, sbs])
                        nc.sync.dma_start(out=xs[:, 4:8, :],
                                          in_=xT3[:, 4:8, sbs])
                    cosv = cos_sb[:, sbs].rearrange("p (a b) -> p a b", a=2)
                    sinv = sin_sb[:, sbs].rearrange("p (a b) -> p a b", a=2)

                    def qk_chain(mode, hp):
                        dst = qt if mode == "q" else kt
                        hps = slice(hp * 128, (hp + 1) * 128)
                        ps = psA.tile([P, 2, 512], FP32, tag="pa", name="pa")
                        for half in range(2):
                            hs = slice(half * 512, (half + 1) * 512)
                            for k in range(KT):
                                nc.tensor.matmul(
                                    ps[:, half, :],
                                    w_sb[mode][:, k, hps],
                                    xs[:, k, hs],
                                    start=(k == 0), stop=(k == KT - 1),
                                )
                        s16 = rpool.tile([P, 2, 512], M16, tag="s16",
                                         name="s16")
                        nc.scalar.copy(s16[:], ps[:, :, :])
                        rot = rpool.tile([P, 2, 512], M16, tag="rot",
                                         name="rot")
                        nc.vector.stream_shuffle(rot[:], s16[:], mask=SHUF16)
                        t1 = rpool.tile([P, 2, 512], M16, tag="t1",
                                        name="t1")
                        nc.vector.tensor_mul(t1[:], s16[:], cosv)
                        t2 = rpool.tile([P, 2, 512], M16, tag="t2",
                                        name="t2")
                        nc.vector.tensor_mul(t2[:], rot[:], sinv)
                        nc.vector.tensor_add(
                            dst[hp][:, sbs].rearrange("p (a b) -> p a b",
                                                      a=2),
                            t1[:], t2[:])

                    def v_chain(stp2):
                        psv = psA.tile([P, 2, 512], FP32, tag="pa",
                                       name="psv")
                        for half in range(2):
                            st_off = slice((stp2 * 2 + half) * 128,
                                           (stp2 * 2 + half + 1) * 128)
                            for k in range(KT):
                                nc.tensor.matmul(
                                    psv[:, half, :],
                                    xs[:, k, st_off],
                                    w_sb["v"][:, k, :],
                                    start=(k == 0), stop=(k == KT - 1),
                                )
                        st0 = sb * 8 + stp2 * 2
                        pv = psv[:, :, :].rearrange(
                            "p a (h d) -> p a h d", h=8)
                        # DVE evict keeps the ACT queue clear for the
                        # hoisted-J0 exps at the phase boundary
                        nc.vector.tensor_copy(v_sb[:, st0:st0 + 2, :, 0:64],
                                              pv[:, :, :, :])

                    if sb == 0:
                        for hp in range(HP):
                            qk_chain("q", hp)
                            qk_chain("k", hp)
                            v_chain(hp)
                    else:
                        # sb1: v chains first (drains the ACT queue backlog
                        # before the exp stream), J0 score slots spliced
                        # between chains to pre-seed the phase-B pipeline
                        for hp in range(HP):
                            v_chain(hp)
                            qk_chain("q", hp)
                            for I in (0, 1):
                                ex, nlo = scores(0, hp, I)
                                pend.append((0, hp, ex, nlo, I))
                            qk_chain("k", hp)
                            for I in (2, 3):
                                ex, nlo = scores(0, hp, I)
                                pend.append((0, hp, ex, nlo, I))

            # ---------------- Phase B: attention ----------------
            ot_pool = ctx.enter_context(tc.tile_pool(name="otp", bufs=1))
            oT = [ot_pool.tile([P, S], MR, tag=f"oT{i}", name=f"oT{i}")
                  for i in range(HP)]

            with ExitStack() as bctx:
                rdpool = bctx.enter_context(tc.tile_pool(name="rdp", bufs=3))
                bcpool = bctx.enter_context(tc.tile_pool(name="bcp", bufs=3))
                opool = bctx.enter_context(tc.tile_pool(name="ostage",
                                                        bufs=2))
                psO = bctx.enter_context(
                    tc.tile_pool(name="psO", bufs=2, space="PSUM"))

                def attnv(po, hp, n_i, ex, nlo, I):
                    nc.tensor.matmul(
                        po[0][0:65, nlo:],
                        v_sb[:, I, hp * 2, :],
                        ex[:, 0, nlo:],
                        start=(I == 0), stop=(I == n_i - 1),
                    )
                    nc.tensor.matmul(
                        po[1][0:65, nlo:],
                        v_sb[:, I, hp * 2 + 1, :],
                        ex[:, 1, nlo:],
                        start=(I == 0), stop=(I == n_i - 1),
                    )

                def normalize(J, hp, po, last=False):
                    Js = slice(J * 512, (J + 1) * 512)
                    if last:
                        bcp = psS.tile([P, 2, 512], FP32, tag="psS",
                                       name="bcp")
                    for h2 in range(2):
                        # den row 64.  32-block stream-transpose spreads it
                        # over 32 partitions, reciprocal hits the one
                        # strided column holding it, transpose back.
                        tb = rdpool.tile([P, 512], FP32, tag="tb",
                                         name="tb")
                        nc.vector.transpose(tb[64:96, :], po[h2][64:96, :])
                        tb2 = rdpool.tile([P, 512], FP32, tag="tb2",
                                          name="tb2")
                        tbv = tb[64:96, :].rearrange("p (b q) -> p b q",
                                                     q=32)
                        tb2v = tb2[64:96, :].rearrange("p (b q) -> p b q",
                                                       q=32)
                        nc.vector.reciprocal(tb2v[:, :, 0:1],
                                             tbv[:, :, 0:1])
                        rd = rdpool.tile([P, 512], FP32, tag="rd",
                                         name="rd")
                        nc.vector.transpose(rd[64:96, :], tb2[64:96, :])
                        if last:
                            nc.tensor.matmul(
                                bcp[0:64, h2, :],
                                ones64[64:65, :],
                                rd[64:65, :],
                                start=True, stop=True,
                            )
                            bcs = bcpool.tile([P, 512], FP32,
                                              tag=f"bc{h2}", name="bcs")
                            nc.scalar.copy(bcs[0:64, :], bcp[0:64, h2, :])
                            bca = bcs[0:64, :]
                        else:
                            nc.sync.dma_start(out=scr[J, hp, h2, :],
                                              in_=rd[64:65, :])
                            bc = bcpool.tile([P, 512], FP32, tag=f"bc{h2}",
                                             name="bc")
                            nc.sync.dma_start(
                                out=bc[0:64, :],
                                in_=scr[J, hp, h2, :].partition_broadcast(
                                    64))
                            bca = bc[0:64, :]
                        if h2 == 0:
                            nc.vector.tensor_mul(
                                oT[hp][0:64, Js],
                                po[0][0:64, :], bca)
                        else:
                            # normalized evict lands at partitions 0-63; a
                            # sync-queue DMA shifts it into oT's upper half
                            tmp = rdpool.tile([P, 512], MR, tag="tmpb",
                                              name="tmpb")
                            nc.vector.tensor_mul(
                                tmp[0:64, :], po[1][0:64, :], bca)
                            nc.sync.dma_start(out=oT[hp][64:128, Js],
                                              in_=tmp[0:64, :])

                def outproj_st(J, i, qsel):
                    st = 4 * J + i
                    stp = slice(st * 128, (st + 1) * 128)
                    pc = psS.tile([P, 2, 512], FP32, tag="psS", name="pc")
                    for nb in range(2):
                        nbs = slice(nb * 512, (nb + 1) * 512)
                        for t in range(4):
                            nc.tensor.matmul(
                                pc[:, nb, :],
                                oT[t][:, stp],
                                wo_sb[:, t, nbs],
                                start=(t == 0), stop=(t == 3),
                            )
                    ob = opool.tile([P, 2, 512], FP32, tag="ob", name="ob")
                    nc.vector.tensor_copy(ob[:], pc[:, :])
                    nc.gpsimd.dma_start(
                        out=outD[stp, :],
                        in_=ob[:, :, :].rearrange("p a b -> p (a b)"))

                # ---- continuous global pipeline over remaining slots ----
                slots = [(J, hp, I)
                         for J in range(1, SB)
                         for hp in range(HP)
                         for I in range(4 * J + 4)]
                po_map = {}
                opq = []
                oq_count = 0

                def pop_attnv():
                    J, hp, ex, nlo, I = pend.pop(0)
                    g = (J, hp)
                    if I == 0:
                        po_map[g] = [psO.tile([P, 512], FP32, tag=f"po{h2}",
                                              name=f"po{h2}")
                                     for h2 in range(2)]
                    attnv(po_map[g], hp, 4 * J + 4, ex, nlo, I)
                    if I == 4 * J + 3:
                        normalize(J, hp, po_map.pop(g),
                                  last=(J == SB - 1 and hp == HP - 1))
                        if hp == HP - 1:
                            opq.extend((J, i) for i in range(4))

                for idx, (J, hp, I) in enumerate(slots):
                    ex, nlo = scores(J, hp, I)
                    pend.append((J, hp, ex, nlo, I))
                    if len(pend) > DEPTH:
                        pop_attnv()
                    if opq and idx % 6 == 5:
                        outproj_st(*opq.pop(0), oq_count)
                        oq_count += 1
                while pend:
                    pop_attnv()
                while opq:
                    outproj_st(*opq.pop(0), oq_count)
                    oq_count += 1

    nc.compile()
    return nc


def get_module(mm="float16"):
    if mm not in _CACHE:
        _CACHE[mm] = _build_module(mm)
    return _CACHE[mm]


def _head_perm():
    """Within-head dim permutation: 16-pair blocks [x1 x2 x1 x2]."""
    p = []
    for blk in range(2):
        base = blk * 32
        p += [2 * (base // 2 + i) for i in range(16)]       # x1 of pairs
        p += [2 * (base // 2 + i) + 1 for i in range(16)]   # x2 of pairs
    return np.array(p)


def prep_core_inputs(inputs, mm="float16"):
    import ml_dtypes
    mdt = {"float16": np.float16, "bfloat16": ml_dtypes.bfloat16}.get(
        mm, np.float32)
    x = np.asarray(inputs["x"], dtype=np.float32)
    tp = np.asarray(inputs["token_positions"])
    wq = np.asarray(inputs["wq"], dtype=np.float32)
    wk = np.asarray(inputs["wk"], dtype=np.float32)
    wv = np.asarray(inputs["wv"], dtype=np.float32)
    wo = np.asarray(inputs["wo"], dtype=np.float32)

    perm = _head_perm()
    qi = np.arange(128)[None, :]
    ki = np.arange(128)[:, None]
    mask01 = np.where(qi < ki, np.float32(0.0),
                      np.float32(1.0)).astype(mdt)

    freqs = 1.0 / THETA ** (np.arange(0, D_K, 2, dtype=np.float32) / D_K)

    in_maps = []
    for c in range(N_CORES):
        b, g = divmod(c, 2)
        rows = slice(g * 512, (g + 1) * 512)
        wq_g = wq[rows].reshape(H_CORE, D_K, D_MODEL)[:, perm, :]
        wk_g = wk[rows].reshape(H_CORE, D_K, D_MODEL)[:, perm, :]

        pos = tp[b].astype(np.float32)
        ang = freqs[:, None] * pos[None, :]          # [32, S]
        cos32, sin32 = np.cos(ang), np.sin(ang)
        # permuted row l: l%32 < 16 -> x1 of pair (l%32 + 16*(l//32)),
        #                 else x2 of the same pair; x1 rows get -sin.
        cos64 = np.concatenate([cos32[0:16], cos32[0:16],
                                cos32[16:32], cos32[16:32]], axis=0)
        sin64 = np.concatenate([-sin32[0:16], sin32[0:16],
                                -sin32[16:32], sin32[16:32]], axis=0)
        cosT = np.tile(cos64, (2, 1))
        sinT = np.tile(sin64, (2, 1))

        in_maps.append({
            "xT": np.ascontiguousarray(x[b].T).astype(mdt),
            "wqT": np.ascontiguousarray(
                wq_g.reshape(512, D_MODEL).T).astype(mdt),
            "wkT": np.ascontiguousarray(
                wk_g.reshape(512, D_MODEL).T).astype(mdt),
            "wvT": np.ascontiguousarray(wv[rows].T).astype(mdt),
            "woT": np.ascontiguousarray(wo[:, rows].T).astype(np.float32),
            "cosT": np.ascontiguousarray(cosT).astype(mdt),
            "sinT": np.ascontiguousarray(sinT).astype(mdt),
            "maskA": mask01,
        })
    return in_maps


DEFAULT_MM = "float16"


def kernel(**inputs):
    from concourse.bass_utils import run_bass_kernel_spmd

    import os
    mm = os.environ.get("KMM", DEFAULT_MM)
    nc = get_module(mm)
    in_maps = prep_core_inputs(inputs, mm)
    res = run_bass_kernel_spmd(nc, in_maps, core_ids=list(range(N_CORES)))
    out = np.empty((B_FULL, S, D_MODEL), np.float32)
    for b in range(B_FULL):
        out[b] = res.results[2 * b]["out"] + res.results[2 * b + 1]["out"]
    return out
